# revision 2
# baseline (speedup 1.0000x reference)
"""Multi-head self-attention (B=4, S=2048, D=1024, H=16) on 8 TRN2 NeuronCores.

Sharding: data parallel over batch (4) x tensor parallel over heads (2 groups
of 8 heads) = 8 cores.

v2 vs baseline:
- All matmuls bf16 (measured: fp8 DoubleRow streams its doubled moving rows
  at the same per-row rate on this silicon, so it buys no throughput; bf16
  minimizes error). 1536 matmul instructions is the structural floor.
- exp split between the Scalar engine (ACT, bf16 out) and DVE Schraudolph
  (one tensor_scalar emitting bf16 bit patterns) per EXP_LUT.
- Softmax epilogue: reciprocal_approx_fast (5x faster than reciprocal) +
  DRAM-bounce partition broadcast; normalize-multiply straight from PSUM.
- DMA emission ordered so the first QKV matmuls start ~6us after launch.
"""

import numpy as np
import ml_dtypes

B, S, D = 4, 2048, 1024
H, HD = 16, 64
P = 128
GF = 512          # features per head-group (8 heads x 64)
QC = 512          # query-chunk (matmul moving free dim)
NQC = S // QC     # 4
KD = D // P       # 8 contraction tiles over d_model
KVT = S // P      # 16 kv tiles
HJ = KVT // 2     # 8 kv-tile pair groups (one exp unit each)

# Schraudolph exp -> bf16 bits: i16 = round(23.0835*s + 16256 + C)
# C = -5.6 minimizes max rel err (3.3%) of the piecewise-linear mantissa
SCH_A = 128 * 0.18034013970657564
SCH_B = 128 * 127 - 5.6

# exp-unit engine assignment, indexed by j within each (qi, h):
# A=Scalar(ACT), D=Vector(DVE Schraudolph). GpSimd cannot read PSUM.
EXP_LUT = "AADAAADA"
# heads 0..NF8-1 run AV as fp8 DoubleRow. Measured on this silicon: a DR
# instruction streams its full 2N moving rows at the same rate as two bf16
# instructions, so DR buys nothing - keep 0 and bank the accuracy.
NF8 = 0

_cache = {}
DEBUG = False


def _build_module():
    import concourse.bacc as bacc
    import concourse.mybir as mybir
    import concourse.tile as tile

    f32 = mybir.dt.float32
    bf16 = mybir.dt.bfloat16
    f8 = mybir.dt.float8e4
    i16 = mybir.dt.int16
    Exp = mybir.ActivationFunctionType.Exp
    DR = mybir.MatmulPerfMode.DoubleRow
    mult = mybir.AluOpType.mult
    add = mybir.AluOpType.add
    subtract = mybir.AluOpType.subtract

    nc = bacc.Bacc("TRN2", target_bir_lowering=False, debug=False)

    xT = nc.dram_tensor("xT", [D, S], bf16, kind="ExternalInput").ap()
    wqT = nc.dram_tensor("wqT", [D, GF], bf16, kind="ExternalInput").ap()
    wkT = nc.dram_tensor("wkT", [D, GF], bf16, kind="ExternalInput").ap()
    wvT = nc.dram_tensor("wvT", [D, GF], bf16, kind="ExternalInput").ap()
    woT = nc.dram_tensor("woT", [GF, D], bf16, kind="ExternalInput").ap()
    bq = nc.dram_tensor("bq", [GF], f32, kind="ExternalInput").ap()
    bk = nc.dram_tensor("bk", [GF], f32, kind="ExternalInput").ap()
    y = nc.dram_tensor("y", [S, D], f32, kind="ExternalOutput").ap()

    dbg = {}
    if DEBUG:
        for nm, shape, dt_ in (
            ("d_qt", [P, QC], bf16), ("d_kt", [P, QC], bf16),
            ("d_s2", [P, 2, QC], f32), ("d_p2a", [P, 2, QC], bf16),
            ("d_p2d", [P, 2, QC], bf16), ("d_av", [HD + 1, QC], f32),
            ("d_bc", [HD, QC], f32), ("d_ot", [P, 4, QC], bf16),
            ("d_vt", [P, 8, HD + 1], bf16),
            ("d_vt1", [P, 8, HD + 1], bf16),
            ("d_vt7", [P, 8, HD + 1], bf16),
            ("d_rc", [1, QC], f32),
        ):
            dbg[nm] = nc.dram_tensor(nm, shape, dt_,
                                     kind="ExternalOutput").ap()

    xT_r = xT.rearrange("(o p) s -> p o s", p=P)     # [128, 8, 2048]
    wq_r = wqT.rearrange("(o p) f -> p o f", p=P)    # [128, 8, 512]
    wk_r = wkT.rearrange("(o p) f -> p o f", p=P)
    wv_r = wvT.rearrange("(o p) f -> p o f", p=P)
    wo_r = woT.rearrange("(o p) d -> p o d", p=P)    # [128, 4, 1024]
    bq_r = bq.rearrange("(o p) -> p o", p=P)         # [128, 4]
    bk_r = bk.rearrange("(o p) -> p o", p=P)

    with tile.TileContext(nc) as tc:
        with tc.tile_pool(name="persist", bufs=1) as persist:
            # q/k pair-stacked (head 2*pair+hh at partition base 64*hh),
            # bf16: fp8 DoubleRow gave no speedup on this silicon (DR does
            # not halve per-row time), so spend the accuracy budget here.
            qt = persist.tile([P, 4, S], bf16, name="qt")
            kt = persist.tile([P, 4, S], bf16, name="kt")
            # V + ones column. fp8 copy for DR heads (kv-pair layout
            # [kv_part, jpair, head, jj, 80(pad)]) + bf16 copy for the rest.
            vt8 = (persist.tile([P, HJ, NF8, 2, 80], f8, name="vt8")
                   if NF8 else None)
            vt16 = persist.tile([P, KVT, 8 - NF8, HD + 1], bf16, name="vt16")
            bq_sb = persist.tile([P, NJ := 4], f32, name="bq_sb")
            bk_sb = persist.tile([P, 4], f32, name="bk_sb")
            if NF8:
                nc.any.memset(vt8[:, :, :, :, HD:HD + 1], 1.0)
            nc.any.memset(vt16[:, :, :, HD:HD + 1], 1.0)
            nc.sync.dma_start(bq_sb[:], bq_r)
            nc.sync.dma_start(bk_sb[:], bk_r)

            # ---------------- QKV projections (bf16) ----------------
            with tc.tile_pool(name="xw", bufs=1) as xw, \
                 tc.tile_pool(name="ps_qkv", bufs=4, space="PSUM") as ps_qkv:
                xt_sb = xw.tile([P, KD, S], bf16, name="xt_sb")
                w_sb = {}
                for nm, w_r in (("q", wq_r), ("k", wk_r), ("v", wv_r)):
                    w_sb[nm] = xw.tile([P, KD, GF], bf16, name=f"w{nm}_sb")
                # order DMAs so the first (qc0, fb*) q-matmuls start early
                for kd in range(KD):
                    nc.sync.dma_start(w_sb["q"][:, kd], wq_r[:, kd])
                for kd in range(KD):
                    nc.sync.dma_start(xt_sb[:, kd, 0:QC], xT_r[:, kd, 0:QC])
                for kd in range(KD):
                    nc.sync.dma_start(xt_sb[:, kd, QC:], xT_r[:, kd, QC:])
                for nm, w_r in (("k", wk_r), ("v", wv_r)):
                    for kd in range(KD):
                        nc.sync.dma_start(w_sb[nm][:, kd], w_r[:, kd])

                for nm, dst, b_sb in (("q", qt, bq_sb), ("k", kt, bk_sb)):
                    for qc in range(NQC):
                        for fb in range(4):
                            pq = ps_qkv.tile([P, QC], f32,
                                             name=f"pq{nm}{fb}{qc}",
                                             tag="qk", bufs=4)
                            for kd in range(KD):
                                nc.tensor.matmul(
                                    pq,
                                    lhsT=w_sb[nm][:, kd, fb * P:(fb + 1) * P],
                                    rhs=xt_sb[:, kd, qc * QC:(qc + 1) * QC],
                                    start=(kd == 0), stop=(kd == KD - 1))
                            csl = slice(qc * QC, (qc + 1) * QC)
                            nc.vector.tensor_scalar_add(
                                dst[:, fb, csl], pq, b_sb[:, fb:fb + 1])

                for t in range(KVT):
                    pv = ps_qkv.tile([P, GF], f32, name=f"pv{t}",
                                     tag="v", bufs=4)
                    for kd in range(KD):
                        nc.tensor.matmul(
                            pv,
                            lhsT=xt_sb[:, kd, t * P:(t + 1) * P],
                            rhs=w_sb["v"][:, kd],
                            start=(kd == 0), stop=(kd == KD - 1))
                    if NF8:
                        nc.vector.tensor_copy(
                            vt8[:, t // 2, :, t % 2, 0:HD],
                            pv[:, 0:NF8 * HD].rearrange("p (h d) -> p h d",
                                                        h=NF8))
                    nc.vector.tensor_copy(
                        vt16[:, t, :, 0:HD],
                        pv[:, NF8 * HD:].rearrange("p (h d) -> p h d",
                                                   h=8 - NF8))

            if DEBUG:
                nc.sync.dma_start(dbg["d_qt"], qt[:, 0, 0:QC])
                nc.sync.dma_start(dbg["d_kt"], kt[:, 0, 0:QC])

            # ---------------- attention + out-projection ----------------
            with tc.tile_pool(name="attn", bufs=2) as attn, \
                 tc.tile_pool(name="wo_pool", bufs=1) as wo_pool, \
                 tc.tile_pool(name="dr", bufs=4, space="DRAM") as dr_pool, \
                 tc.tile_pool(name="ps_s", bufs=2, space="PSUM") as ps_s, \
                 tc.tile_pool(name="ps_av", bufs=2, space="PSUM") as ps_av, \
                 tc.tile_pool(name="ps_yo", bufs=2, space="PSUM") as ps_yo:
                wo_sb = wo_pool.tile([P, 4, D], bf16, name="wo_sb")
                for fb in range(4):
                    nc.sync.dma_start(wo_sb[:, fb], wo_r[:, fb])

                ot_tiles = {}

                def out_proj_chunk(qi, sq):
                    ot_t = ot_tiles[qi]
                    y_t = attn.tile([P, D], f32, name=f"y{qi}{sq}",
                                    tag="y", bufs=3)
                    for dm in range(2):
                        yps = ps_yo.tile([P, QC], f32, name=f"yp{qi}{sq}{dm}",
                                         tag="yo", bufs=2)
                        for fb in range(4):
                            nc.tensor.matmul(
                                yps,
                                lhsT=ot_t[:, fb, sq * P:(sq + 1) * P],
                                rhs=wo_sb[:, fb, dm * QC:(dm + 1) * QC],
                                start=(fb == 0), stop=(fb == 3))
                        nc.vector.tensor_copy(y_t[:, dm * QC:(dm + 1) * QC],
                                              yps)
                    row0 = qi * QC + sq * P
                    nc.sync.dma_start(y[row0:row0 + P, :], y_t[:])

                def emit_exp(p2, s2, j, is8=False):
                    eng = "A" if is8 else EXP_LUT[j % len(EXP_LUT)]
                    if eng == "A":
                        nc.scalar.activation(p2[:], s2[:], Exp, scale=0.125)
                    else:
                        e = nc.vector if eng == "D" else nc.gpsimd
                        e.tensor_scalar(
                            out=p2[:].bitcast(i16), in0=s2[:],
                            scalar1=SCH_A, scalar2=SCH_B,
                            op0=mult, op1=add)

                for qi in range(NQC):
                    qsl = slice(qi * QC, (qi + 1) * QC)
                    ot_t = attn.tile([P, 4, QC], bf16, name=f"ot{qi}",
                                     tag="ot", bufs=2)
                    ot_tiles[qi] = ot_t
                    for h in range(8):
                        pair, hh = h // 2, h % 2
                        psl = slice(64 * hh, 64 * hh + 64)
                        avp = ps_av.tile([P, QC], f32, name=f"av{qi}{h}",
                                         tag="av", bufs=2)[:HD + 1]

                        p2_prev = {}
                        is8 = h < NF8

                        def emit_av(j, p2_t):
                            if is8:
                                nc.tensor.matmul(
                                    avp,
                                    lhsT=vt8[:, j, h, :, 0:HD + 1],
                                    rhs=p2_t[:],
                                    start=(j == 0), stop=(j == HJ - 1),
                                    perf_mode=DR)
                            else:
                                for jj in range(2):
                                    kvt = 2 * j + jj
                                    nc.tensor.matmul(
                                        avp,
                                        lhsT=vt16[:, kvt, h - NF8],
                                        rhs=p2_t[:, jj],
                                        start=(kvt == 0),
                                        stop=(kvt == KVT - 1))

                        # AV lags scores by 2 kv-pair groups so the PE never
                        # head-of-line blocks on the exp engines.
                        for j in range(HJ):
                            s2 = ps_s.tile([P, 2, QC], f32,
                                           name=f"s{qi}{h}{j}", tag="s",
                                           bufs=2)
                            for jj in range(2):
                                kvt = 2 * j + jj
                                nc.tensor.matmul(
                                    s2[:, jj],
                                    lhsT=kt[psl, pair, kvt * P:(kvt + 1) * P],
                                    rhs=qt[psl, pair, qsl],
                                    start=True, stop=True)
                            p2 = attn.tile([P, 2, QC], f8 if is8 else bf16,
                                           name=f"p{qi}{h}{j}", tag="p",
                                           bufs=6)
                            if DEBUG and qi == 0 and h == 0 and j == 0:
                                s_sb = attn.tile([P, 2, QC], f32,
                                                 name="dbg_s", tag="dbg",
                                                 bufs=1)
                                nc.vector.tensor_copy(s_sb[:], s2[:])
                                nc.sync.dma_start(dbg["d_s2"], s_sb[:])
                            emit_exp(p2, s2, j, is8)
                            if DEBUG and qi == 0 and h == 0 and j == 0:
                                nc.sync.dma_start(dbg["d_p2a"], p2[:])
                            if DEBUG and qi == 0 and h == 0 and j == 2:
                                nc.sync.dma_start(dbg["d_p2d"], p2[:])
                            p2_prev[j] = p2
                            if j >= 2:
                                emit_av(j - 2, p2_prev.pop(j - 2))
                        for j in (HJ - 2, HJ - 1):
                            emit_av(j, p2_prev.pop(j))

                        if DEBUG and qi == 0 and h == 0:
                            av_sb = attn.tile([HD + 1, QC], f32,
                                              name="dbg_av", tag="dbg_av",
                                              bufs=1)
                            nc.vector.tensor_copy(av_sb[:], avp[:])
                            nc.sync.dma_start(dbg["d_av"], av_sb[:])
                        # epilogue: approx-reciprocal of the denominator row,
                        # DRAM-bounce broadcast, normalize straight from PSUM
                        # reciprocal_approx_fast misreads PSUM; stage the
                        # denominator row in SBUF first
                        den_sb = attn.tile([1, QC], f32, name=f"dn{qi}{h}",
                                           tag="den", bufs=4)
                        nc.vector.tensor_copy(den_sb[:], avp[HD:HD + 1])
                        recip = attn.tile([1, QC], f32, name=f"r{qi}{h}",
                                          tag="recip", bufs=4)
                        nc.vector.reciprocal_approx_fast(
                            recip[:], den_sb[:])
                        r_dr = dr_pool.tile([1, QC], f32, name=f"rd{qi}{h}",
                                            tag="rd", bufs=4)
                        nc.sync.dma_start(r_dr[:], recip[:])
                        bc_sb = attn.tile([HD, QC], f32, name=f"bc{qi}{h}",
                                          tag="bc", bufs=4)
                        nc.sync.dma_start(
                            bc_sb[:], r_dr[:].to_broadcast((HD, QC)))
                        # ot feature f = 128*fb + p, natural order f = 64h+hd
                        # => fb = pair, partition half = hh
                        if hh == 0:
                            nc.vector.tensor_mul(
                                ot_t[0:HD, pair], avp[:HD], bc_sb[:])
                        else:
                            # DVE can't write partition-shifted; bounce
                            # through SBUF->SBUF DMA to land on 64..127.
                            otmp = attn.tile([HD, QC], bf16,
                                             name=f"ox{qi}{h}",
                                             tag="otmp", bufs=2)
                            nc.vector.tensor_mul(
                                otmp[:], avp[:HD], bc_sb[:])
                            nc.sync.dma_start(ot_t[HD:P, pair], otmp[:])
                        if DEBUG and qi == 0 and h == 0:
                            nc.sync.dma_start(dbg["d_rc"], recip[:])
                            nc.sync.dma_start(dbg["d_bc"], bc_sb[:])
                        if qi > 0 and h < 4:
                            out_proj_chunk(qi - 1, h)
                    if DEBUG and qi == 0:
                        nc.sync.dma_start(dbg["d_ot"], ot_t[:])
                for sq in range(4):
                    out_proj_chunk(NQC - 1, sq)

    nc.compile()
    return nc
